# revision 19
# baseline (speedup 1.0000x reference)
"""Trainium2 Bass kernel: Diffusion-DEM PINN total loss (data-parallel, 8 cores).

Strategy v2 (fp16 + term-folding)
---------------------------------
Pure data parallel over 8 cores; small MLP replicated. Forward-mode
derivative propagation (channels v, d/dx, d2/dx2, d3/dx3, d/dt) with:

- All elementwise chain math in fp16 SBUF tiles at W=1024 width (DVE 2x mode).
- Matmuls in fp16 (lhsT + rhs), fp32 PSUM accumulation, N=512 blocks.
- Term folding: h_xx / h_xxx are never materialized. The chain-rule additive
  terms (pb2 = p*z2, s1 = u1*a, pz3 = p*z3, v1 = u1*b2, v2 = w1*q) are fed
  separately to the next layer's matmuls, which accumulate them in PSUM with
  constant scales folded into weight copies (W, -2W, -6W).
- Layer-1 derivative constants (w0x, w0t powers) are folded into precomputed
  weight variants diag(.)@W1, so layer 1 only computes p0, q0, t1=y0*p0,
  t2=p0*q0.
- Output projections W3 are packed as zero-padded [128, 8] stationary tiles
  accumulating all quantities (u,ux,uxx,uxxx,c,cx,cxx,ct) into one [8,512]
  PSUM tile per block; staged to DRAM in the same row layout the (unchanged)
  fp32 residual phase expects.

Channel order in staging rows: u,ux,uxx,uxxx,c,cx,cxx,ct (interior),
u,ux,uxx,c,cx (boundary), u,c (init).
"""

import numpy as np
from contextlib import ExitStack

import concourse.bass as bass
import concourse.tile as tile
import concourse.mybir as mybir
from concourse import bacc
from concourse.bass_utils import run_bass_kernel_spmd

F32 = mybir.dt.float32
F16 = mybir.dt.float16
AF = mybir.ActivationFunctionType
OP = mybir.AluOpType

M = 8                      # cores
N_INT, N_BND, H = 65536, 8192, 128
NI = N_INT // M            # 8192 interior pts / core
NB = N_BND // M            # 1024 bnd / init pts / core
B = 512                    # matmul / PSUM block
W = 1024                   # fp16 SBUF chain width (2 blocks)
NBI, NBB = NI // B, NB // B           # 16, 2
NUI, NUB = NI // W, NB // W           # 8, 1 units
PI, PB = NI // 128, NB // 128         # 64, 8 point-major free dims

# physical constants (from reference)
NU = 0.3
ALFA = 0.001 * 8e-07 * 3.497e-06 / 7.08e-15
THETA = 3.497e-06 * 1e10 / (8.3145 * 300.0 * 3.0 * 0.4)
LAM1 = 1.0 / (1.0 - 2.0 * NU)
LAM2 = (1.0 - NU) / (1.0 + NU)
LAM3 = NU / (1.0 + NU)
K1 = ALFA * 1.3 / (0.7 * 3.0)

OUT_COLS = 10  # fs2, resid2, stress_l2, fd_l2, stress_r2, fd_r2, iu0, iu1, ic0, ic1


def as_ap(v):
    return v if isinstance(v, bass.AP) else v[:]


def emit(nc, tc, ctx, io):
    const = ctx.enter_context(tc.tile_pool(name="const", bufs=1))
    chan = ctx.enter_context(tc.tile_pool(name="chan", bufs=3))
    stg = ctx.enter_context(tc.tile_pool(name="stg", bufs=3))
    res = ctx.enter_context(tc.tile_pool(name="res", bufs=1))
    ps_a = ctx.enter_context(tc.tile_pool(name="ps_a", bufs=3, space="PSUM"))
    ps_b = ctx.enter_context(tc.tile_pool(name="ps_b", bufs=2, space="PSUM"))
    dram = ctx.enter_context(tc.tile_pool(name="dram", bufs=1, space="DRAM"))

    def load_const(name, shape, dt=F32):
        t = const.tile(list(shape), dt, tag=name, name=name)
        nc.sync.dma_start(t[:], io[name][:])
        return t

    # inputs (x split hi/lo fp16 for exactness)
    Xi4 = load_const("Xi4", (4, NI), F16)
    Xl4 = load_const("Xl4", (4, NB), F16)
    Xr4 = load_const("Xr4", (4, NB), F16)
    X04 = load_const("X04", (4, NB), F16)
    # layer-1 folded weights
    W0n4 = load_const("W0n4", (4, H), F16)
    b0n = load_const("b0n", (H, 1))
    W1v = load_const("W1v", (H, H), F16)
    W1x = load_const("W1x", (H, H), F16)
    W1t = load_const("W1t", (H, H), F16)
    W1xx = load_const("W1xx", (H, H), F16)   # diag(-2 w0x^2) @ W1
    W1xxx = load_const("W1xxx", (H, H), F16)  # diag(-2 w0x^3) @ W1
    b1 = load_const("b1", (H, 1))
    W2 = load_const("W2", (H, H), F16)
    W2m2 = load_const("W2m2", (H, H), F16)   # -2 W2
    W2m6 = load_const("W2m6", (H, H), F16)   # -6 W2
    b2 = load_const("b2", (H, 1))
    # output projection variants [128, 8]
    w3i = {k: load_const("W3i_" + k, (H, 8), F16)
           for k in ("v", "x", "xx", "xxm2", "u3", "u3m6", "u3m2", "t")}
    w3b = {k: load_const("W3b_" + k, (H, 8), F16)
           for k in ("v", "x", "xx", "xxm2")}
    w30 = {"v": load_const("W30_v", (H, 8), F16)}
    # residual-phase constants
    b3u = load_const("b3u", (128, 1))
    b3c = load_const("b3c", (128, 1))
    x_pm = load_const("x_pm", (128, PI))
    xl_pm = load_const("xl_pm", (128, PB))
    xr_pm = load_const("xr_pm", (128, PB))
    Wi0 = load_const("Wi0", (128, 2, PB))
    Yi0 = load_const("Yi0", (128, 2, PB))

    Ud_i = dram.tile([8, NI], F32, tag="Ud_i")
    Ud_l = dram.tile([5, NB], F32, tag="Ud_l")
    Ud_r = dram.tile([5, NB], F32, tag="Ud_r")
    Ud_0 = dram.tile([2, NB], F32, tag="Ud_0")

    def ct(shape, tag):
        return chan.tile(list(shape), F16, tag=tag, name=tag)

    def layer1(X4, u, nch):
        """Unit u of point set. Returns dict of L1 output tiles (fp16, W wide)."""
        y0 = ct((128, W), "y0")
        for k in range(W // B):
            zv = ps_a.tile([128, B], F32, tag="zv", name="zv")
            col = slice(u * W + k * B, u * W + (k + 1) * B)
            nc.tensor.matmul(zv[:], lhsT=W0n4[:], rhs=X4[:, col])
            nc.scalar.activation(y0[:, k * B:(k + 1) * B], zv[:], AF.Tanh,
                                 bias=b0n[:], scale=1.0)
        ysq = ct((128, W), "ysq0")
        nc.scalar.activation(ysq[:], y0[:], AF.Square)
        p0 = ct((128, W), "p0")
        nc.vector.tensor_scalar(p0[:], ysq[:], -1.0, 1.0, OP.mult, OP.add)
        t1 = ct((128, W), "t10")
        nc.vector.tensor_mul(t1[:], y0[:], p0[:])
        out = {"v": y0, "x": p0, "t": p0, "xx": t1}
        if nch == 5:
            q0 = ct((128, W), "q0")
            nc.vector.tensor_scalar(q0[:], ysq[:], -3.0, 1.0, OP.mult, OP.add)
            t2 = ct((128, W), "t20")
            nc.vector.tensor_mul(t2[:], p0[:], q0[:])
            out["xxx"] = t2
        return out

    def z_mms_from_l1(c, nch, k):
        """Matmuls layer1 outputs -> layer2 z tiles for block k of the unit.
        Returns (zv, z12, z3t) PSUM tiles."""
        col = slice(k * B, (k + 1) * B)
        zv = ps_a.tile([128, B], F32, tag="zv", name="zv")
        nc.tensor.matmul(zv[:], lhsT=W1v[:], rhs=c["v"][:, col])
        z12 = ps_b.tile([128, 2 * B], F32, tag="zz", name="z12")
        nc.tensor.matmul(z12[:, 0:B], lhsT=W1x[:], rhs=c["x"][:, col])
        nc.tensor.matmul(z12[:, B:2 * B], lhsT=W1xx[:], rhs=c["xx"][:, col])
        z3t = None
        if nch == 5:
            z3t = ps_b.tile([128, 2 * B], F32, tag="zz", name="z3t")
            nc.tensor.matmul(z3t[:, 0:B], lhsT=W1xxx[:], rhs=c["xxx"][:, col])
            nc.tensor.matmul(z3t[:, B:2 * B], lhsT=W1t[:], rhs=c["t"][:, col])
        return zv, z12, z3t

    def z_mms_from_hidden(c, nch, k):
        """Matmuls hidden-layer term tiles -> next z tiles for block k."""
        col = slice(k * B, (k + 1) * B)
        zv = ps_a.tile([128, B], F32, tag="zv", name="zv")
        nc.tensor.matmul(zv[:], lhsT=W2[:], rhs=c["v"][:, col])
        z12 = ps_b.tile([128, 2 * B], F32, tag="zz", name="z12")
        nc.tensor.matmul(z12[:, 0:B], lhsT=W2[:], rhs=c["x"][:, col])
        nc.tensor.matmul(z12[:, B:2 * B], lhsT=W2[:], rhs=c["pb2"][:, col],
                         start=True, stop=False)
        nc.tensor.matmul(z12[:, B:2 * B], lhsT=W2m2[:], rhs=c["s1"][:, col],
                         start=False, stop=True)
        z3t = None
        if nch == 5:
            z3t = ps_b.tile([128, 2 * B], F32, tag="zz", name="z3t")
            nc.tensor.matmul(z3t[:, 0:B], lhsT=W2[:], rhs=c["pz3"][:, col],
                             start=True, stop=False)
            nc.tensor.matmul(z3t[:, 0:B], lhsT=W2m6[:], rhs=c["v1"][:, col],
                             start=False, stop=False)
            nc.tensor.matmul(z3t[:, 0:B], lhsT=W2m2[:], rhs=c["v2"][:, col],
                             start=False, stop=True)
            nc.tensor.matmul(z3t[:, B:2 * B], lhsT=W2[:], rhs=c["t"][:, col])
        return zv, z12, z3t

    def hidden(z_mms, src, nch, li):
        """One hidden tanh layer with derivative term propagation.
        z_mms(src, nch, k) emits the matmuls for block k; src is the previous
        layer's tile dict. Returns this layer's term-tile dict."""
        bias = b1 if li == 1 else b2
        y = ct((128, W), "hy")
        T12 = ct((128, 2, W), "T12")   # a | b2
        T3t = ct((128, 2, W), "T3t") if nch == 5 else None
        for k in range(W // B):
            zv, z12, z3t = z_mms(src, nch, k)
            nc.scalar.activation(y[:, k * B:(k + 1) * B], zv[:], AF.Tanh,
                                 bias=bias[:], scale=1.0)
            nc.scalar.activation(
                T12[:, :, k * B:(k + 1) * B],
                z12[:].rearrange("p (r c) -> p r c", r=2), AF.Copy)
            if nch == 5:
                nc.scalar.activation(
                    T3t[:, :, k * B:(k + 1) * B],
                    z3t[:].rearrange("p (r c) -> p r c", r=2), AF.Copy)
        a, b2s = T12[:, 0, :], T12[:, 1, :]
        ysq = ct((128, W), "hysq")
        nc.scalar.activation(ysq[:], y[:], AF.Square)
        p = ct((128, W), "hp")
        nc.vector.tensor_scalar(p[:], ysq[:], -1.0, 1.0, OP.mult, OP.add)
        hx = ct((128, W), "hhx")
        nc.vector.tensor_tensor(hx[:], p[:], a, OP.mult)
        u1 = ct((128, W), "hu1")
        nc.vector.tensor_mul(u1[:], y[:], hx[:])
        s1 = ct((128, W), "hs1")
        nc.vector.tensor_tensor(s1[:], u1[:], a, OP.mult)
        pb2 = ct((128, W), "hpb2")
        nc.vector.tensor_tensor(pb2[:], p[:], b2s, OP.mult)
        out = {"v": y, "x": hx, "s1": s1, "pb2": pb2}
        if nch == 5:
            c3, d = T3t[:, 0, :], T3t[:, 1, :]
            q = ct((128, W), "hq")
            nc.vector.tensor_scalar(q[:], ysq[:], -3.0, 1.0, OP.mult, OP.add)
            v1 = ct((128, W), "hv1")
            nc.vector.tensor_tensor(v1[:], u1[:], b2s, OP.mult)
            asq = ct((128, W), "hasq")
            nc.scalar.activation(asq[:], a, AF.Square)
            w1 = ct((128, W), "hw1")
            nc.vector.tensor_mul(w1[:], hx[:], asq[:])
            v2 = ct((128, W), "hv2")
            nc.vector.tensor_mul(v2[:], w1[:], q[:])
            pz3 = ct((128, W), "hpz3")
            nc.vector.tensor_tensor(pz3[:], p[:], c3, OP.mult)
            ht = ct((128, W), "hht")
            nc.vector.tensor_tensor(ht[:], p[:], d, OP.mult)
            out.update({"pz3": pz3, "v1": v1, "v2": v2, "t": ht})
        return out

    def proj_and_stage(c, nch, Ud, u, rows):
        """Project last hidden layer terms by W3 variants, accumulate in PSUM,
        stage [rows, B] fp32 to DRAM Ud."""
        if nch == 5:
            w3, terms = w3i, (("v", "v"), ("x", "x"), ("xx", "pb2"),
                              ("xxm2", "s1"), ("u3", "pz3"), ("u3m6", "v1"),
                              ("u3m2", "v2"), ("t", "t"))
        elif nch == 3:
            w3, terms = w3b, (("v", "v"), ("x", "x"), ("xx", "pb2"),
                              ("xxm2", "s1"))
        else:
            w3, terms = w30, (("v", "v"),)
        for k in range(W // B):
            col = slice(k * B, (k + 1) * B)
            po = ps_a.tile([128, B], F32, tag="zv", name="po")
            o = po[0:8, :]
            for ti, (wk, ck) in enumerate(terms):
                nc.tensor.matmul(o, lhsT=w3[wk][:], rhs=c[ck][:, col],
                                 start=ti == 0, stop=ti == len(terms) - 1)
            ob = stg.tile([8, B], F32, tag="ob", name="ob")
            if k % 2 == 0:
                nc.scalar.activation(ob[0:rows, :], po[0:rows, :], AF.Copy)
            else:
                nc.vector.tensor_copy(ob[0:rows, :], po[0:rows, :])
            nc.sync.dma_start(Ud[0:rows, u * W + k * B: u * W + (k + 1) * B],
                              ob[0:rows, :])

    def emit_unit(X4, u, nch, Ud, rows):
        c = layer1(X4, u, nch)
        if nch == 1:
            # value-only chain (init set)
            y = c["v"]
            for li, (Wm, bias) in enumerate(((W1v, b1), (W2, b2)), start=1):
                y2 = ct((128, W), "hy")
                for k in range(W // B):
                    zv = ps_a.tile([128, B], F32, tag="zv", name="zv")
                    nc.tensor.matmul(zv[:], lhsT=Wm[:],
                                     rhs=y[:, k * B:(k + 1) * B])
                    nc.scalar.activation(y2[:, k * B:(k + 1) * B], zv[:],
                                         AF.Tanh, bias=bias[:], scale=1.0)
                y = y2
            proj_and_stage({"v": y}, 1, Ud, u, rows)
            return
        c = hidden(z_mms_from_l1, c, nch, 1)
        c = hidden(z_mms_from_hidden, c, nch, 2)
        proj_and_stage(c, nch, Ud, u, rows)

    # ---------------- schedule ----------------
    jobs = []
    small = [("l", 0), ("r", 0), ("0", 0)]
    for u in range(NUI):
        jobs.append(("i", u))
        if u % 3 == 2 and small:
            jobs.append(small.pop(0))
    jobs += small
    for set_, u in jobs:
        if set_ == "i":
            emit_unit(Xi4, u, 5, Ud_i, 8)
        elif set_ == "l":
            emit_unit(Xl4, u, 3, Ud_l, 5)
        elif set_ == "r":
            emit_unit(Xr4, u, 3, Ud_r, 5)
        else:
            emit_unit(X04, u, 1, Ud_0, 2)

    # ---------------- residual phase (fp32, as baseline) ----------------
    out_sb = res.tile([128, OUT_COLS], F32, tag="out_sb")

    def rt(tag, w=PI):
        return res.tile([128, w], F32, tag=tag, name=tag)

    def tt(out, a, b_, op=OP.mult):
        nc.vector.tensor_tensor(as_ap(out), as_ap(a), as_ap(b_), op)
        return out

    def stt(out, in0, scal, in1, op0=OP.mult, op1=OP.add):
        nc.vector.scalar_tensor_tensor(as_ap(out), as_ap(in0), scal,
                                       as_ap(in1), op0, op1)
        return out

    def sumsq(src, colidx, scale=1.0, w=PI):
        dump = res.tile([128, w], F32, tag="dump", name="dump")
        nc.vector.tensor_tensor(dump[:], as_ap(src), as_ap(src), OP.mult)
        nc.vector.tensor_scalar(
            dump[:], dump[:], float(scale), 0.0, OP.mult, OP.add,
            accum_out=out_sb[:, colidx:colidx + 1])

    bnd_stage = {"l": Ud_l, "r": Ud_r}
    for si, (setname, xp, rhs) in enumerate((("l", xl_pm, 0.0), ("r", xr_pm, 1.0))):
        Ub = res.tile([128, 5, PB], F32, tag="Ub", name="Ub")
        nc.sync.dma_start(Ub[:], bnd_stage[setname][:].rearrange(
            "c (p i) -> p c i", p=128))
        ub_ = res.tile([128, PB], F32, tag="bu", name="bu")
        nc.vector.tensor_scalar(ub_[:], Ub[:, 0, :], b3u[:], None, OP.add)
        cb = res.tile([128, PB], F32, tag="bc", name="bc")
        nc.vector.tensor_scalar(cb[:], Ub[:, 3, :], b3c[:], None, OP.add)
        uxb, uxxb, cxb = Ub[:, 1, :], Ub[:, 2, :], Ub[:, 4, :]
        bx2 = res.tile([128, PB], F32, tag="bx2", name="bx2")
        nc.scalar.activation(bx2[:], xp[:], AF.Square)
        rx = res.tile([128, PB], F32, tag="rx", name="rx")
        nc.vector.reciprocal(rx[:], xp[:])
        t1 = res.tile([128, PB], F32, tag="t1b", name="t1b")
        tt(t1, ub_, rx)
        t2 = res.tile([128, PB], F32, tag="t2b", name="t2b")
        nc.vector.tensor_scalar(t2[:], t1[:], LAM3, None, OP.mult)
        t3 = res.tile([128, PB], F32, tag="t3b", name="t3b")
        nc.vector.scalar_tensor_tensor(t3[:], uxb, LAM2, t2[:], OP.mult, OP.add)
        t4 = res.tile([128, PB], F32, tag="t4b", name="t4b")
        nc.vector.scalar_tensor_tensor(t4[:], cb[:], -ALFA / 3.0, t3[:],
                                       OP.mult, OP.add)
        sumsq(t4, 2 + 2 * si, scale=LAM1 * LAM1, w=PB)
        m1b = res.tile([128, PB], F32, tag="m1b", name="m1b")
        nc.vector.tensor_tensor(m1b[:], bx2[:], uxxb, OP.mult)
        m2b = res.tile([128, PB], F32, tag="m2b", name="m2b")
        nc.vector.tensor_tensor(m2b[:], xp[:], uxb, OP.mult)
        m3b = res.tile([128, PB], F32, tag="m3b", name="m3b")
        tt(m3b, m1b, m2b, OP.add)
        m4b = res.tile([128, PB], F32, tag="m4b", name="m4b")
        tt(m4b, m3b, ub_, OP.subtract)
        Cb = res.tile([128, PB], F32, tag="Cb", name="Cb")
        nc.vector.tensor_tensor(Cb[:], bx2[:], cxb, OP.mult)
        m5b = res.tile([128, PB], F32, tag="m5b", name="m5b")
        stt(m5b, Cb, -ALFA, m4b)
        m6b = res.tile([128, PB], F32, tag="m6b", name="m6b")
        tt(m6b, cb, m5b)
        fd = res.tile([128, PB], F32, tag="fd", name="fd")
        stt(fd, m6b, -THETA, Cb)
        if rhs != 0.0:
            fd2 = res.tile([128, PB], F32, tag="fd2", name="fd2")
            tt(fd2, fd, bx2, OP.subtract)
            fd = fd2
        sumsq(fd, 3 + 2 * si, w=PB)

    # init residuals
    U0 = res.tile([128, 2, PB], F32, tag="U0")
    nc.sync.dma_start(U0[:], Ud_0[:].rearrange("c (p i) -> p c i", p=128))
    iu = res.tile([128, PB], F32, tag="iu")
    nc.vector.tensor_scalar(iu[:], U0[:, 0, :], b3u[:], None, OP.add)
    ic = res.tile([128, PB], F32, tag="ic")
    nc.vector.tensor_scalar(ic[:], U0[:, 1, :], b3c[:], None, OP.add)
    for oi, val in enumerate((iu, ic)):
        for j in range(2):
            d = res.tile([128, PB], F32, tag="d0", name="d0")
            nc.vector.tensor_tensor(d[:], val[:], Yi0[:, j, :], OP.subtract)
            dw = res.tile([128, PB], F32, tag="dw", name="dw")
            nc.vector.tensor_tensor(dw[:], d[:], Wi0[:, j, :], OP.mult)
            dump = res.tile([128, PB], F32, tag="dump0", name="dump0")
            nc.vector.tensor_tensor(dump[:], d[:], dw[:], OP.mult)
            nc.vector.tensor_scalar(
                dump[:], dump[:], 1.0, 0.0, OP.mult, OP.add,
                accum_out=out_sb[:, 6 + 2 * oi + j:7 + 2 * oi + j])

    # interior residuals
    Ui = res.tile([128, 8, PI], F32, tag="Ui")
    nc.sync.dma_start(Ui[:], Ud_i[:].rearrange("c (p i) -> p c i", p=128))
    ub_t = rt("ub_t")
    nc.vector.tensor_scalar(ub_t[:], Ui[:, 0, :], b3u[:], None, OP.add)
    cb_t = rt("cb_t")
    nc.vector.tensor_scalar(cb_t[:], Ui[:, 4, :], b3c[:], None, OP.add)
    x = x_pm
    x2 = rt("x2")
    nc.scalar.activation(x2[:], x[:], AF.Square)
    x3 = tt(rt("x3"), x2, x)
    A = tt(rt("A"), x2[:], Ui[:, 2, :])          # x2*uxx
    Bt = tt(rt("Bt"), x[:], Ui[:, 1, :])         # x*ux
    C = tt(rt("C"), x2[:], Ui[:, 5, :])          # x2*cx
    D = tt(rt("D"), x3[:], Ui[:, 6, :])          # x3*cxx
    E = tt(rt("E"), x3[:], Ui[:, 3, :])          # x3*uxxx
    F = tt(rt("F"), x3[:], Ui[:, 7, :])          # x3*ct
    j1 = tt(rt("j1"), A, Bt, OP.add)
    j2 = tt(rt("j2"), j1, ub_t, OP.subtract)     # A+B-u
    fs = stt(rt("fs"), C, -K1, j2)               # -K1*C + j2
    sumsq(fs, 0)
    in2 = stt(rt("in2"), C, -ALFA, j2)           # inner2
    i1 = stt(rt("i1"), A, 2.0, Bt, OP.mult, OP.subtract)   # 2A - B
    i2 = tt(rt("i2"), i1, ub_t, OP.add)
    i3 = stt(rt("i3"), C, -ALFA, E)
    i4 = tt(rt("i4"), i2, i3, OP.add)
    i5 = stt(rt("i5"), D, -ALFA, i4)             # inner1
    k1 = tt(rt("k1"), cb_t, i5)
    k2 = tt(rt("k2"), x, Ui[:, 5, :])            # x*cx
    k3 = tt(rt("k3"), k2, in2)
    k4 = stt(rt("k4"), k1, THETA, F)
    k5 = stt(rt("k5"), k3, THETA, k4)
    k7 = tt(rt("k7"), D, C, OP.add)
    r_ = tt(rt("r_"), k5, k7, OP.subtract)
    sumsq(r_, 1)

    nc.sync.dma_start(io["out"][:], out_sb[:])


def build_nc():
    nc = bacc.Bacc("TRN2", target_bir_lowering=False, debug=False, num_devices=M)
    io = {}

    def dp(name, shape, is_out=False, dt=F32):
        h = nc.declare_dram_parameter(name, list(shape), dt, isOutput=is_out)
        io[name] = h.ap()

    dp("Xi4", (4, NI), dt=F16); dp("Xl4", (4, NB), dt=F16)
    dp("Xr4", (4, NB), dt=F16); dp("X04", (4, NB), dt=F16)
    dp("W0n4", (4, H), dt=F16); dp("b0n", (H, 1))
    for n in ("W1v", "W1x", "W1t", "W1xx", "W1xxx", "W2", "W2m2", "W2m6"):
        dp(n, (H, H), dt=F16)
    dp("b1", (H, 1)); dp("b2", (H, 1))
    for n in ("v", "x", "xx", "xxm2", "u3", "u3m6", "u3m2", "t"):
        dp("W3i_" + n, (H, 8), dt=F16)
    for n in ("v", "x", "xx", "xxm2"):
        dp("W3b_" + n, (H, 8), dt=F16)
    dp("W30_v", (H, 8), dt=F16)
    dp("b3u", (128, 1)); dp("b3c", (128, 1))
    dp("x_pm", (128, PI)); dp("xl_pm", (128, PB)); dp("xr_pm", (128, PB))
    dp("Wi0", (128, 2, PB)); dp("Yi0", (128, 2, PB))
    dp("out", (128, OUT_COLS), is_out=True)

    with tile.TileContext(nc) as tc:
        with ExitStack() as ctx:
            emit(nc, tc, ctx, io)
    nc.compile()
    return nc


def host_prep(inputs):
    """Fold normalization + layer-1 derivative constants into weights; build
    the 8 per-core input maps."""
    f4, f2 = np.float32, np.float16
    g = {k: np.asarray(v) for k, v in inputs.items()}
    Xint = g["Xint"].astype(f4)
    lb = Xint.min(axis=0).astype(np.float64)
    ub = Xint.max(axis=0).astype(np.float64)
    a = 2.0 / (ub - lb)
    beta = -2.0 * lb / (ub - lb) - 1.0
    W0 = g["W0"].astype(np.float64)
    W0n = a[:, None] * W0                      # [2, H]
    b0n = beta @ W0 + g["b0"].astype(np.float64)
    w0x, w0t = W0n[0], W0n[1]
    W0n4 = np.stack([W0n[0], W0n[0], W0n[1], W0n[1]])   # [4, H] (xhi,xlo,thi,tlo)

    W1 = g["W1"].astype(np.float64)
    W2 = g["W2"].astype(np.float64)
    W3 = g["W3"].astype(np.float64)            # [H, 2]

    def w3p(cols_vals):
        """Zero-padded [H, 8] projection matrix: {col: H-vector}."""
        m = np.zeros((H, 8), np.float64)
        for c, v in cols_vals.items():
            m[:, c] = v
        return m.astype(f2)

    # interior/bnd staging rows: u,ux,uxx,uxxx,c,cx,cxx,ct (bnd uses 0..4)
    com = {
        "W0n4": W0n4.astype(f2),
        "b0n": b0n.astype(f4).reshape(H, 1),
        "W1v": W1.astype(f2),
        "W1x": (w0x[:, None] * W1).astype(f2),
        "W1t": (w0t[:, None] * W1).astype(f2),
        "W1xx": (-2.0 * w0x[:, None] ** 2 * W1).astype(f2),
        "W1xxx": (-2.0 * w0x[:, None] ** 3 * W1).astype(f2),
        "b1": g["b1"].astype(f4).reshape(H, 1),
        "W2": W2.astype(f2),
        "W2m2": (-2.0 * W2).astype(f2),
        "W2m6": (-6.0 * W2).astype(f2),
        "b2": g["b2"].astype(f4).reshape(H, 1),
        # interior rows: u,ux,uxx,uxxx,c,cx,cxx,ct
        "W3i_v": w3p({0: W3[:, 0], 4: W3[:, 1]}),
        "W3i_x": w3p({1: W3[:, 0], 5: W3[:, 1]}),
        "W3i_xx": w3p({2: W3[:, 0], 6: W3[:, 1]}),
        "W3i_xxm2": w3p({2: -2.0 * W3[:, 0], 6: -2.0 * W3[:, 1]}),
        "W3i_u3": w3p({3: W3[:, 0]}),
        "W3i_u3m6": w3p({3: -6.0 * W3[:, 0]}),
        "W3i_u3m2": w3p({3: -2.0 * W3[:, 0]}),
        "W3i_t": w3p({7: W3[:, 1]}),
        # boundary rows: u,ux,uxx,c,cx
        "W3b_v": w3p({0: W3[:, 0], 3: W3[:, 1]}),
        "W3b_x": w3p({1: W3[:, 0], 4: W3[:, 1]}),
        "W3b_xx": w3p({2: W3[:, 0]}),
        "W3b_xxm2": w3p({2: -2.0 * W3[:, 0]}),
        # init rows: u, c
        "W30_v": w3p({0: W3[:, 0], 1: W3[:, 1]}),
        "b3u": np.full((128, 1), g["b3"][0], f4),
        "b3c": np.full((128, 1), g["b3"][1], f4),
    }

    def split16(v):
        hi = v.astype(f2)
        lo = (v.astype(np.float64) - hi.astype(np.float64)).astype(f2)
        return hi, lo

    in_maps = []
    for ci in range(M):
        Xi = Xint[ci * NI:(ci + 1) * NI]
        Xl = g["Xbnd_l"][ci * NB:(ci + 1) * NB].astype(f4)
        Xr = g["Xbnd_r"][ci * NB:(ci + 1) * NB].astype(f4)
        X0 = g["Xinit"][ci * NB:(ci + 1) * NB].astype(f4)
        Wi = g["Winit"][ci * NB:(ci + 1) * NB].astype(f4)
        Yi = g["Yinit"][ci * NB:(ci + 1) * NB].astype(f4)
        m = dict(com)

        def x4(X):
            xh, xl = split16(X[:, 0])
            th, tl = split16(X[:, 1])
            return np.ascontiguousarray(np.stack([xh, xl, th, tl]))

        m["Xi4"] = x4(Xi)
        m["Xl4"] = x4(Xl)
        m["Xr4"] = x4(Xr)
        m["X04"] = x4(X0)
        m["x_pm"] = np.ascontiguousarray(Xi[:, 0].reshape(128, PI))
        m["xl_pm"] = np.ascontiguousarray(Xl[:, 0].reshape(128, PB))
        m["xr_pm"] = np.ascontiguousarray(Xr[:, 0].reshape(128, PB))
        m["Wi0"] = np.ascontiguousarray(Wi.reshape(128, PB, 2).transpose(0, 2, 1))
        m["Yi0"] = np.ascontiguousarray(Yi.reshape(128, PB, 2).transpose(0, 2, 1))
        in_maps.append(m)
    return in_maps


def combine(results):
    s = np.zeros(OUT_COLS, np.float64)
    for r in results:
        s += r["out"].astype(np.float64).sum(axis=0)
    int_loss = (s[0] + s[1]) / N_INT
    bnd_loss = (s[2] + s[3]) / N_BND + (s[4] + s[5]) / N_BND
    init_loss = (s[6] + s[7] + s[8] + s[9]) / (2 * N_BND)
    return np.float32(int_loss + bnd_loss + init_loss)


_CACHE = {}


def _get_nc():
    if "nc" not in _CACHE:
        _CACHE["nc"] = build_nc()
    return _CACHE["nc"]


def kernel(**inputs):
    in_maps = host_prep(inputs)
    nc = _get_nc()
    res = run_bass_kernel_spmd(nc, in_maps, core_ids=list(range(M)))
    return combine(res.results)


# revision 20
# speedup vs baseline: 1.2836x; 1.2836x over previous
"""Trainium2 Bass kernel: Diffusion-DEM PINN total loss (data-parallel, 8 cores).

Strategy v2 (fp16 + term-folding)
---------------------------------
Pure data parallel over 8 cores; small MLP replicated. Forward-mode
derivative propagation (channels v, d/dx, d2/dx2, d3/dx3, d/dt) with:

- All elementwise chain math in fp16 SBUF tiles at W=1024 width (DVE 2x mode).
- Matmuls in fp16 (lhsT + rhs), fp32 PSUM accumulation, N=512 blocks.
- Term folding: h_xx / h_xxx are never materialized. The chain-rule additive
  terms (pb2 = p*z2, s1 = u1*a, pz3 = p*z3, v1 = u1*b2, v2 = w1*q) are fed
  separately to the next layer's matmuls, which accumulate them in PSUM with
  constant scales folded into weight copies (W, -2W, -6W).
- Layer-1 derivative constants (w0x, w0t powers) are folded into precomputed
  weight variants diag(.)@W1, so layer 1 only computes p0, q0, t1=y0*p0,
  t2=p0*q0.
- Output projections W3 are packed as zero-padded [128, 8] stationary tiles
  accumulating all quantities (u,ux,uxx,uxxx,c,cx,cxx,ct) into one [8,512]
  PSUM tile per block; staged to DRAM in the same row layout the (unchanged)
  fp32 residual phase expects.

Channel order in staging rows: u,ux,uxx,uxxx,c,cx,cxx,ct (interior),
u,ux,uxx,c,cx (boundary), u,c (init).
"""

import numpy as np
from contextlib import ExitStack

import concourse.bass as bass
import concourse.tile as tile
import concourse.mybir as mybir
from concourse import bacc
from concourse.bass_utils import run_bass_kernel_spmd

F32 = mybir.dt.float32
F16 = mybir.dt.float16
AF = mybir.ActivationFunctionType
OP = mybir.AluOpType

M = 8                      # cores
N_INT, N_BND, H = 65536, 8192, 128
NI = N_INT // M            # 8192 interior pts / core
NB = N_BND // M            # 1024 bnd / init pts / core
B = 512                    # matmul / PSUM block
W = 1024                   # fp16 SBUF chain width (2 blocks)
NBI, NBB = NI // B, NB // B           # 16, 2
NUI, NUB = NI // W, NB // W           # 8, 1 units
PI, PB = NI // 128, NB // 128         # 64, 8 point-major free dims

# physical constants (from reference)
NU = 0.3
ALFA = 0.001 * 8e-07 * 3.497e-06 / 7.08e-15
THETA = 3.497e-06 * 1e10 / (8.3145 * 300.0 * 3.0 * 0.4)
LAM1 = 1.0 / (1.0 - 2.0 * NU)
LAM2 = (1.0 - NU) / (1.0 + NU)
LAM3 = NU / (1.0 + NU)
K1 = ALFA * 1.3 / (0.7 * 3.0)

OUT_COLS = 10  # fs2, resid2, stress_l2, fd_l2, stress_r2, fd_r2, iu0, iu1, ic0, ic1


def as_ap(v):
    return v if isinstance(v, bass.AP) else v[:]


def emit(nc, tc, ctx, io):
    const = ctx.enter_context(tc.tile_pool(name="const", bufs=1))
    chan = ctx.enter_context(tc.tile_pool(name="chan", bufs=2))
    stg = ctx.enter_context(tc.tile_pool(name="stg", bufs=3))
    res = ctx.enter_context(tc.tile_pool(name="res", bufs=1))
    ps_a = ctx.enter_context(tc.tile_pool(name="ps_a", bufs=3, space="PSUM"))
    ps_b = ctx.enter_context(tc.tile_pool(name="ps_b", bufs=2, space="PSUM"))
    dram = ctx.enter_context(tc.tile_pool(name="dram", bufs=1, space="DRAM"))

    def load_const(name, shape, dt=F32):
        t = const.tile(list(shape), dt, tag=name, name=name)
        nc.sync.dma_start(t[:], io[name][:])
        return t

    # inputs (x split hi/lo fp16 for exactness)
    Xi4 = load_const("Xi4", (4, NI), F16)
    Xl4 = load_const("Xl4", (4, NB), F16)
    Xr4 = load_const("Xr4", (4, NB), F16)
    X04 = load_const("X04", (4, NB), F16)
    # layer-1 folded weights
    W0n4 = load_const("W0n4", (4, H), F16)
    b0n = load_const("b0n", (H, 1))
    W1v = load_const("W1v", (H, H), F16)
    W1x = load_const("W1x", (H, H), F16)
    W1t = load_const("W1t", (H, H), F16)
    W1xx = load_const("W1xx", (H, H), F16)   # diag(-2 w0x^2) @ W1
    W1xxx = load_const("W1xxx", (H, H), F16)  # diag(-2 w0x^3) @ W1
    b1 = load_const("b1", (H, 1))
    W2 = load_const("W2", (H, H), F16)
    W2m2 = load_const("W2m2", (H, H), F16)   # -2 W2
    W2m6 = load_const("W2m6", (H, H), F16)   # -6 W2
    b2 = load_const("b2", (H, 1))
    # output projection variants [128, 8]
    w3i = {k: load_const("W3i_" + k, (H, 8), F16)
           for k in ("v", "x", "xx", "xxm2", "u3", "u3m6", "u3m2", "t")}
    w3b = {k: load_const("W3b_" + k, (H, 8), F16)
           for k in ("v", "x", "xx", "xxm2")}
    w30 = {"v": load_const("W30_v", (H, 8), F16)}
    # residual-phase constants
    b3u = load_const("b3u", (128, 1))
    b3c = load_const("b3c", (128, 1))
    x_pm = load_const("x_pm", (128, PI))
    xl_pm = load_const("xl_pm", (128, PB))
    xr_pm = load_const("xr_pm", (128, PB))
    Wi0 = load_const("Wi0", (128, 2, PB))
    Yi0 = load_const("Yi0", (128, 2, PB))

    Ud_i = dram.tile([8, NI], F32, tag="Ud_i")
    Ud_l = dram.tile([5, NB], F32, tag="Ud_l")
    Ud_r = dram.tile([5, NB], F32, tag="Ud_r")
    Ud_0 = dram.tile([2, NB], F32, tag="Ud_0")

    def ct(shape, tag):
        return chan.tile(list(shape), F16, tag=tag, name=tag)

    def layer1(X4, u, nch):
        """Unit u of point set. Returns dict of L1 output tiles (fp16, W wide)."""
        y0 = ct((128, W), "y0")
        for k in range(W // B):
            zv = ps_a.tile([128, B], F32, tag="zv", name="zv")
            col = slice(u * W + k * B, u * W + (k + 1) * B)
            nc.tensor.matmul(zv[:], lhsT=W0n4[:], rhs=X4[:, col])
            nc.scalar.activation(y0[:, k * B:(k + 1) * B], zv[:], AF.Tanh,
                                 bias=b0n[:], scale=1.0)
        ysq = ct((128, W), "ysq0")
        nc.scalar.activation(ysq[:], y0[:], AF.Square)
        p0 = ct((128, W), "p0")
        nc.vector.tensor_scalar(p0[:], ysq[:], -1.0, 1.0, OP.mult, OP.add)
        t1 = ct((128, W), "t10")
        nc.vector.tensor_mul(t1[:], y0[:], p0[:])
        out = {"v": y0, "x": p0, "t": p0, "xx": t1}
        if nch == 5:
            q0 = ct((128, W), "q0")
            nc.vector.tensor_scalar(q0[:], ysq[:], -3.0, 1.0, OP.mult, OP.add)
            t2 = ct((128, W), "t20")
            nc.vector.tensor_mul(t2[:], p0[:], q0[:])
            out["xxx"] = t2
        return out

    def z_mms_from_l1(c, nch, k):
        """Matmuls layer1 outputs -> layer2 z tiles for block k of the unit.
        Returns (zv, z12, z3t) PSUM tiles."""
        col = slice(k * B, (k + 1) * B)
        zv = ps_a.tile([128, B], F32, tag="zv", name="zv")
        nc.tensor.matmul(zv[:], lhsT=W1v[:], rhs=c["v"][:, col])
        z12 = ps_b.tile([128, 2 * B], F32, tag="zz", name="z12")
        nc.tensor.matmul(z12[:, 0:B], lhsT=W1x[:], rhs=c["x"][:, col])
        nc.tensor.matmul(z12[:, B:2 * B], lhsT=W1xx[:], rhs=c["xx"][:, col])
        z3t = None
        if nch == 5:
            z3t = ps_b.tile([128, 2 * B], F32, tag="zz", name="z3t")
            nc.tensor.matmul(z3t[:, 0:B], lhsT=W1xxx[:], rhs=c["xxx"][:, col])
            nc.tensor.matmul(z3t[:, B:2 * B], lhsT=W1t[:], rhs=c["t"][:, col])
        return zv, z12, z3t

    def z_mms_from_hidden(c, nch, k):
        """Matmuls hidden-layer term tiles -> next z tiles for block k."""
        col = slice(k * B, (k + 1) * B)
        zv = ps_a.tile([128, B], F32, tag="zv", name="zv")
        nc.tensor.matmul(zv[:], lhsT=W2[:], rhs=c["v"][:, col])
        z12 = ps_b.tile([128, 2 * B], F32, tag="zz", name="z12")
        nc.tensor.matmul(z12[:, 0:B], lhsT=W2[:], rhs=c["x"][:, col])
        nc.tensor.matmul(z12[:, B:2 * B], lhsT=W2[:], rhs=c["pb2"][:, col],
                         start=True, stop=False)
        nc.tensor.matmul(z12[:, B:2 * B], lhsT=W2m2[:], rhs=c["s1"][:, col],
                         start=False, stop=True)
        z3t = None
        if nch == 5:
            z3t = ps_b.tile([128, 2 * B], F32, tag="zz", name="z3t")
            nc.tensor.matmul(z3t[:, 0:B], lhsT=W2[:], rhs=c["pz3"][:, col],
                             start=True, stop=False)
            nc.tensor.matmul(z3t[:, 0:B], lhsT=W2m6[:], rhs=c["v1"][:, col],
                             start=False, stop=False)
            nc.tensor.matmul(z3t[:, 0:B], lhsT=W2m2[:], rhs=c["v2"][:, col],
                             start=False, stop=True)
            nc.tensor.matmul(z3t[:, B:2 * B], lhsT=W2[:], rhs=c["t"][:, col])
        return zv, z12, z3t

    def hidden(z_mms, src, nch, li):
        """One hidden tanh layer with derivative term propagation.
        z_mms(src, nch, k) emits the matmuls for block k; src is the previous
        layer's tile dict. Returns this layer's term-tile dict."""
        bias = b1 if li == 1 else b2
        y = ct((128, W), "hy")
        T12 = ct((128, 2, W), "T12")   # a | b2
        T3t = ct((128, 2, W), "T3t") if nch == 5 else None
        for k in range(W // B):
            zv, z12, z3t = z_mms(src, nch, k)
            nc.scalar.activation(y[:, k * B:(k + 1) * B], zv[:], AF.Tanh,
                                 bias=bias[:], scale=1.0)
            nc.scalar.activation(
                T12[:, :, k * B:(k + 1) * B],
                z12[:].rearrange("p (r c) -> p r c", r=2), AF.Copy)
            if nch == 5:
                nc.scalar.activation(
                    T3t[:, :, k * B:(k + 1) * B],
                    z3t[:].rearrange("p (r c) -> p r c", r=2), AF.Copy)
        a, b2s = T12[:, 0, :], T12[:, 1, :]
        ysq = ct((128, W), "hysq")
        nc.vector.tensor_mul(ysq[:], y[:], y[:])
        p = ct((128, W), "hp")
        nc.vector.tensor_scalar(p[:], ysq[:], -1.0, 1.0, OP.mult, OP.add)
        hx = ct((128, W), "hhx")
        nc.vector.tensor_tensor(hx[:], p[:], a, OP.mult)
        u1 = ct((128, W), "hu1")
        nc.vector.tensor_mul(u1[:], y[:], hx[:])
        s1 = ct((128, W), "hs1")
        nc.vector.tensor_tensor(s1[:], u1[:], a, OP.mult)
        pb2 = ct((128, W), "hpb2")
        nc.vector.tensor_tensor(pb2[:], p[:], b2s, OP.mult)
        out = {"v": y, "x": hx, "s1": s1, "pb2": pb2}
        if nch == 5:
            c3, d = T3t[:, 0, :], T3t[:, 1, :]
            q = ct((128, W), "hq")
            nc.vector.tensor_scalar(q[:], ysq[:], -3.0, 1.0, OP.mult, OP.add)
            v1 = ct((128, W), "hv1")
            nc.vector.tensor_tensor(v1[:], u1[:], b2s, OP.mult)
            asq = ct((128, W), "hasq")
            nc.scalar.activation(asq[:], a, AF.Square)
            w1 = ct((128, W), "hw1")
            nc.vector.tensor_mul(w1[:], hx[:], asq[:])
            v2 = ct((128, W), "hv2")
            nc.vector.tensor_mul(v2[:], w1[:], q[:])
            pz3 = ct((128, W), "hpz3")
            nc.vector.tensor_tensor(pz3[:], p[:], c3, OP.mult)
            ht = ct((128, W), "hht")
            nc.vector.tensor_tensor(ht[:], p[:], d, OP.mult)
            out.update({"pz3": pz3, "v1": v1, "v2": v2, "t": ht})
        return out

    def proj_and_stage(c, nch, Ud, u, rows):
        """Project last hidden layer terms by W3 variants, accumulate in PSUM,
        stage [rows, B] fp32 to DRAM Ud."""
        if nch == 5:
            w3, terms = w3i, (("v", "v"), ("x", "x"), ("xx", "pb2"),
                              ("xxm2", "s1"), ("u3", "pz3"), ("u3m6", "v1"),
                              ("u3m2", "v2"), ("t", "t"))
        elif nch == 3:
            w3, terms = w3b, (("v", "v"), ("x", "x"), ("xx", "pb2"),
                              ("xxm2", "s1"))
        else:
            w3, terms = w30, (("v", "v"),)
        for k in range(W // B):
            col = slice(k * B, (k + 1) * B)
            po = ps_a.tile([128, B], F32, tag="zv", name="po")
            o = po[0:8, :]
            for ti, (wk, ck) in enumerate(terms):
                nc.tensor.matmul(o, lhsT=w3[wk][:], rhs=c[ck][:, col],
                                 start=ti == 0, stop=ti == len(terms) - 1)
            ob = stg.tile([8, B], F32, tag="ob", name="ob")
            if k % 2 == 0:
                nc.scalar.activation(ob[0:rows, :], po[0:rows, :], AF.Copy)
            else:
                nc.vector.tensor_copy(ob[0:rows, :], po[0:rows, :])
            nc.sync.dma_start(Ud[0:rows, u * W + k * B: u * W + (k + 1) * B],
                              ob[0:rows, :])

    def emit_unit(X4, u, nch, Ud, rows):
        c = layer1(X4, u, nch)
        if nch == 1:
            # value-only chain (init set)
            y = c["v"]
            for li, (Wm, bias) in enumerate(((W1v, b1), (W2, b2)), start=1):
                y2 = ct((128, W), "hy")
                for k in range(W // B):
                    zv = ps_a.tile([128, B], F32, tag="zv", name="zv")
                    nc.tensor.matmul(zv[:], lhsT=Wm[:],
                                     rhs=y[:, k * B:(k + 1) * B])
                    nc.scalar.activation(y2[:, k * B:(k + 1) * B], zv[:],
                                         AF.Tanh, bias=bias[:], scale=1.0)
                y = y2
            proj_and_stage({"v": y}, 1, Ud, u, rows)
            return
        c = hidden(z_mms_from_l1, c, nch, 1)
        c = hidden(z_mms_from_hidden, c, nch, 2)
        proj_and_stage(c, nch, Ud, u, rows)

    # ---------------- schedule ----------------
    jobs = []
    small = [("l", 0), ("r", 0), ("0", 0)]
    for u in range(NUI):
        jobs.append(("i", u))
        if u % 3 == 2 and small:
            jobs.append(small.pop(0))
    jobs += small
    for set_, u in jobs:
        if set_ == "i":
            emit_unit(Xi4, u, 5, Ud_i, 8)
        elif set_ == "l":
            emit_unit(Xl4, u, 3, Ud_l, 5)
        elif set_ == "r":
            emit_unit(Xr4, u, 3, Ud_r, 5)
        else:
            emit_unit(X04, u, 1, Ud_0, 2)

    # ---------------- residual phase (fp32, as baseline) ----------------
    out_sb = res.tile([128, OUT_COLS], F32, tag="out_sb")

    def rt(tag, w=PI):
        return res.tile([128, w], F32, tag=tag, name=tag)

    def tt(out, a, b_, op=OP.mult):
        nc.vector.tensor_tensor(as_ap(out), as_ap(a), as_ap(b_), op)
        return out

    def stt(out, in0, scal, in1, op0=OP.mult, op1=OP.add):
        nc.vector.scalar_tensor_tensor(as_ap(out), as_ap(in0), scal,
                                       as_ap(in1), op0, op1)
        return out

    def sumsq(src, colidx, scale=1.0, w=PI):
        dump = res.tile([128, w], F32, tag="dump", name="dump")
        nc.vector.tensor_tensor(dump[:], as_ap(src), as_ap(src), OP.mult)
        nc.vector.tensor_scalar(
            dump[:], dump[:], float(scale), 0.0, OP.mult, OP.add,
            accum_out=out_sb[:, colidx:colidx + 1])

    bnd_stage = {"l": Ud_l, "r": Ud_r}
    for si, (setname, xp, rhs) in enumerate((("l", xl_pm, 0.0), ("r", xr_pm, 1.0))):
        Ub = res.tile([128, 5, PB], F32, tag="Ub", name="Ub")
        nc.sync.dma_start(Ub[:], bnd_stage[setname][:].rearrange(
            "c (p i) -> p c i", p=128))
        ub_ = res.tile([128, PB], F32, tag="bu", name="bu")
        nc.vector.tensor_scalar(ub_[:], Ub[:, 0, :], b3u[:], None, OP.add)
        cb = res.tile([128, PB], F32, tag="bc", name="bc")
        nc.vector.tensor_scalar(cb[:], Ub[:, 3, :], b3c[:], None, OP.add)
        uxb, uxxb, cxb = Ub[:, 1, :], Ub[:, 2, :], Ub[:, 4, :]
        bx2 = res.tile([128, PB], F32, tag="bx2", name="bx2")
        nc.scalar.activation(bx2[:], xp[:], AF.Square)
        rx = res.tile([128, PB], F32, tag="rx", name="rx")
        nc.vector.reciprocal(rx[:], xp[:])
        t1 = res.tile([128, PB], F32, tag="t1b", name="t1b")
        tt(t1, ub_, rx)
        t2 = res.tile([128, PB], F32, tag="t2b", name="t2b")
        nc.vector.tensor_scalar(t2[:], t1[:], LAM3, None, OP.mult)
        t3 = res.tile([128, PB], F32, tag="t3b", name="t3b")
        nc.vector.scalar_tensor_tensor(t3[:], uxb, LAM2, t2[:], OP.mult, OP.add)
        t4 = res.tile([128, PB], F32, tag="t4b", name="t4b")
        nc.vector.scalar_tensor_tensor(t4[:], cb[:], -ALFA / 3.0, t3[:],
                                       OP.mult, OP.add)
        sumsq(t4, 2 + 2 * si, scale=LAM1 * LAM1, w=PB)
        m1b = res.tile([128, PB], F32, tag="m1b", name="m1b")
        nc.vector.tensor_tensor(m1b[:], bx2[:], uxxb, OP.mult)
        m2b = res.tile([128, PB], F32, tag="m2b", name="m2b")
        nc.vector.tensor_tensor(m2b[:], xp[:], uxb, OP.mult)
        m3b = res.tile([128, PB], F32, tag="m3b", name="m3b")
        tt(m3b, m1b, m2b, OP.add)
        m4b = res.tile([128, PB], F32, tag="m4b", name="m4b")
        tt(m4b, m3b, ub_, OP.subtract)
        Cb = res.tile([128, PB], F32, tag="Cb", name="Cb")
        nc.vector.tensor_tensor(Cb[:], bx2[:], cxb, OP.mult)
        m5b = res.tile([128, PB], F32, tag="m5b", name="m5b")
        stt(m5b, Cb, -ALFA, m4b)
        m6b = res.tile([128, PB], F32, tag="m6b", name="m6b")
        tt(m6b, cb, m5b)
        fd = res.tile([128, PB], F32, tag="fd", name="fd")
        stt(fd, m6b, -THETA, Cb)
        if rhs != 0.0:
            fd2 = res.tile([128, PB], F32, tag="fd2", name="fd2")
            tt(fd2, fd, bx2, OP.subtract)
            fd = fd2
        sumsq(fd, 3 + 2 * si, w=PB)

    # init residuals
    U0 = res.tile([128, 2, PB], F32, tag="U0")
    nc.sync.dma_start(U0[:], Ud_0[:].rearrange("c (p i) -> p c i", p=128))
    iu = res.tile([128, PB], F32, tag="iu")
    nc.vector.tensor_scalar(iu[:], U0[:, 0, :], b3u[:], None, OP.add)
    ic = res.tile([128, PB], F32, tag="ic")
    nc.vector.tensor_scalar(ic[:], U0[:, 1, :], b3c[:], None, OP.add)
    for oi, val in enumerate((iu, ic)):
        for j in range(2):
            d = res.tile([128, PB], F32, tag="d0", name="d0")
            nc.vector.tensor_tensor(d[:], val[:], Yi0[:, j, :], OP.subtract)
            dw = res.tile([128, PB], F32, tag="dw", name="dw")
            nc.vector.tensor_tensor(dw[:], d[:], Wi0[:, j, :], OP.mult)
            dump = res.tile([128, PB], F32, tag="dump0", name="dump0")
            nc.vector.tensor_tensor(dump[:], d[:], dw[:], OP.mult)
            nc.vector.tensor_scalar(
                dump[:], dump[:], 1.0, 0.0, OP.mult, OP.add,
                accum_out=out_sb[:, 6 + 2 * oi + j:7 + 2 * oi + j])

    # interior residuals
    Ui = res.tile([128, 8, PI], F32, tag="Ui")
    nc.sync.dma_start(Ui[:], Ud_i[:].rearrange("c (p i) -> p c i", p=128))
    ub_t = rt("ub_t")
    nc.vector.tensor_scalar(ub_t[:], Ui[:, 0, :], b3u[:], None, OP.add)
    cb_t = rt("cb_t")
    nc.vector.tensor_scalar(cb_t[:], Ui[:, 4, :], b3c[:], None, OP.add)
    x = x_pm
    x2 = rt("x2")
    nc.scalar.activation(x2[:], x[:], AF.Square)
    x3 = tt(rt("x3"), x2, x)
    A = tt(rt("A"), x2[:], Ui[:, 2, :])          # x2*uxx
    Bt = tt(rt("Bt"), x[:], Ui[:, 1, :])         # x*ux
    C = tt(rt("C"), x2[:], Ui[:, 5, :])          # x2*cx
    D = tt(rt("D"), x3[:], Ui[:, 6, :])          # x3*cxx
    E = tt(rt("E"), x3[:], Ui[:, 3, :])          # x3*uxxx
    F = tt(rt("F"), x3[:], Ui[:, 7, :])          # x3*ct
    j1 = tt(rt("j1"), A, Bt, OP.add)
    j2 = tt(rt("j2"), j1, ub_t, OP.subtract)     # A+B-u
    fs = stt(rt("fs"), C, -K1, j2)               # -K1*C + j2
    sumsq(fs, 0)
    in2 = stt(rt("in2"), C, -ALFA, j2)           # inner2
    i1 = stt(rt("i1"), A, 2.0, Bt, OP.mult, OP.subtract)   # 2A - B
    i2 = tt(rt("i2"), i1, ub_t, OP.add)
    i3 = stt(rt("i3"), C, -ALFA, E)
    i4 = tt(rt("i4"), i2, i3, OP.add)
    i5 = stt(rt("i5"), D, -ALFA, i4)             # inner1
    k1 = tt(rt("k1"), cb_t, i5)
    k2 = tt(rt("k2"), x, Ui[:, 5, :])            # x*cx
    k3 = tt(rt("k3"), k2, in2)
    k4 = stt(rt("k4"), k1, THETA, F)
    k5 = stt(rt("k5"), k3, THETA, k4)
    k7 = tt(rt("k7"), D, C, OP.add)
    r_ = tt(rt("r_"), k5, k7, OP.subtract)
    sumsq(r_, 1)

    nc.sync.dma_start(io["out"][:], out_sb[:])


def build_nc():
    nc = bacc.Bacc("TRN2", target_bir_lowering=False, debug=False, num_devices=M)
    io = {}

    def dp(name, shape, is_out=False, dt=F32):
        h = nc.declare_dram_parameter(name, list(shape), dt, isOutput=is_out)
        io[name] = h.ap()

    dp("Xi4", (4, NI), dt=F16); dp("Xl4", (4, NB), dt=F16)
    dp("Xr4", (4, NB), dt=F16); dp("X04", (4, NB), dt=F16)
    dp("W0n4", (4, H), dt=F16); dp("b0n", (H, 1))
    for n in ("W1v", "W1x", "W1t", "W1xx", "W1xxx", "W2", "W2m2", "W2m6"):
        dp(n, (H, H), dt=F16)
    dp("b1", (H, 1)); dp("b2", (H, 1))
    for n in ("v", "x", "xx", "xxm2", "u3", "u3m6", "u3m2", "t"):
        dp("W3i_" + n, (H, 8), dt=F16)
    for n in ("v", "x", "xx", "xxm2"):
        dp("W3b_" + n, (H, 8), dt=F16)
    dp("W30_v", (H, 8), dt=F16)
    dp("b3u", (128, 1)); dp("b3c", (128, 1))
    dp("x_pm", (128, PI)); dp("xl_pm", (128, PB)); dp("xr_pm", (128, PB))
    dp("Wi0", (128, 2, PB)); dp("Yi0", (128, 2, PB))
    dp("out", (128, OUT_COLS), is_out=True)

    with tile.TileContext(nc) as tc:
        with ExitStack() as ctx:
            emit(nc, tc, ctx, io)
    nc.compile()
    return nc


def host_prep(inputs):
    """Fold normalization + layer-1 derivative constants into weights; build
    the 8 per-core input maps."""
    f4, f2 = np.float32, np.float16
    g = {k: np.asarray(v) for k, v in inputs.items()}
    Xint = g["Xint"].astype(f4)
    lb = Xint.min(axis=0).astype(np.float64)
    ub = Xint.max(axis=0).astype(np.float64)
    a = 2.0 / (ub - lb)
    beta = -2.0 * lb / (ub - lb) - 1.0
    W0 = g["W0"].astype(np.float64)
    W0n = a[:, None] * W0                      # [2, H]
    b0n = beta @ W0 + g["b0"].astype(np.float64)
    w0x, w0t = W0n[0], W0n[1]
    W0n4 = np.stack([W0n[0], W0n[0], W0n[1], W0n[1]])   # [4, H] (xhi,xlo,thi,tlo)

    W1 = g["W1"].astype(np.float64)
    W2 = g["W2"].astype(np.float64)
    W3 = g["W3"].astype(np.float64)            # [H, 2]

    def w3p(cols_vals):
        """Zero-padded [H, 8] projection matrix: {col: H-vector}."""
        m = np.zeros((H, 8), np.float64)
        for c, v in cols_vals.items():
            m[:, c] = v
        return m.astype(f2)

    # interior/bnd staging rows: u,ux,uxx,uxxx,c,cx,cxx,ct (bnd uses 0..4)
    com = {
        "W0n4": W0n4.astype(f2),
        "b0n": b0n.astype(f4).reshape(H, 1),
        "W1v": W1.astype(f2),
        "W1x": (w0x[:, None] * W1).astype(f2),
        "W1t": (w0t[:, None] * W1).astype(f2),
        "W1xx": (-2.0 * w0x[:, None] ** 2 * W1).astype(f2),
        "W1xxx": (-2.0 * w0x[:, None] ** 3 * W1).astype(f2),
        "b1": g["b1"].astype(f4).reshape(H, 1),
        "W2": W2.astype(f2),
        "W2m2": (-2.0 * W2).astype(f2),
        "W2m6": (-6.0 * W2).astype(f2),
        "b2": g["b2"].astype(f4).reshape(H, 1),
        # interior rows: u,ux,uxx,uxxx,c,cx,cxx,ct
        "W3i_v": w3p({0: W3[:, 0], 4: W3[:, 1]}),
        "W3i_x": w3p({1: W3[:, 0], 5: W3[:, 1]}),
        "W3i_xx": w3p({2: W3[:, 0], 6: W3[:, 1]}),
        "W3i_xxm2": w3p({2: -2.0 * W3[:, 0], 6: -2.0 * W3[:, 1]}),
        "W3i_u3": w3p({3: W3[:, 0]}),
        "W3i_u3m6": w3p({3: -6.0 * W3[:, 0]}),
        "W3i_u3m2": w3p({3: -2.0 * W3[:, 0]}),
        "W3i_t": w3p({7: W3[:, 1]}),
        # boundary rows: u,ux,uxx,c,cx
        "W3b_v": w3p({0: W3[:, 0], 3: W3[:, 1]}),
        "W3b_x": w3p({1: W3[:, 0], 4: W3[:, 1]}),
        "W3b_xx": w3p({2: W3[:, 0]}),
        "W3b_xxm2": w3p({2: -2.0 * W3[:, 0]}),
        # init rows: u, c
        "W30_v": w3p({0: W3[:, 0], 1: W3[:, 1]}),
        "b3u": np.full((128, 1), g["b3"][0], f4),
        "b3c": np.full((128, 1), g["b3"][1], f4),
    }

    def split16(v):
        hi = v.astype(f2)
        lo = (v.astype(np.float64) - hi.astype(np.float64)).astype(f2)
        return hi, lo

    in_maps = []
    for ci in range(M):
        Xi = Xint[ci * NI:(ci + 1) * NI]
        Xl = g["Xbnd_l"][ci * NB:(ci + 1) * NB].astype(f4)
        Xr = g["Xbnd_r"][ci * NB:(ci + 1) * NB].astype(f4)
        X0 = g["Xinit"][ci * NB:(ci + 1) * NB].astype(f4)
        Wi = g["Winit"][ci * NB:(ci + 1) * NB].astype(f4)
        Yi = g["Yinit"][ci * NB:(ci + 1) * NB].astype(f4)
        m = dict(com)

        def x4(X):
            xh, xl = split16(X[:, 0])
            th, tl = split16(X[:, 1])
            return np.ascontiguousarray(np.stack([xh, xl, th, tl]))

        m["Xi4"] = x4(Xi)
        m["Xl4"] = x4(Xl)
        m["Xr4"] = x4(Xr)
        m["X04"] = x4(X0)
        m["x_pm"] = np.ascontiguousarray(Xi[:, 0].reshape(128, PI))
        m["xl_pm"] = np.ascontiguousarray(Xl[:, 0].reshape(128, PB))
        m["xr_pm"] = np.ascontiguousarray(Xr[:, 0].reshape(128, PB))
        m["Wi0"] = np.ascontiguousarray(Wi.reshape(128, PB, 2).transpose(0, 2, 1))
        m["Yi0"] = np.ascontiguousarray(Yi.reshape(128, PB, 2).transpose(0, 2, 1))
        in_maps.append(m)
    return in_maps


def combine(results):
    s = np.zeros(OUT_COLS, np.float64)
    for r in results:
        s += r["out"].astype(np.float64).sum(axis=0)
    int_loss = (s[0] + s[1]) / N_INT
    bnd_loss = (s[2] + s[3]) / N_BND + (s[4] + s[5]) / N_BND
    init_loss = (s[6] + s[7] + s[8] + s[9]) / (2 * N_BND)
    return np.float32(int_loss + bnd_loss + init_loss)


_CACHE = {}


def _get_nc():
    if "nc" not in _CACHE:
        _CACHE["nc"] = build_nc()
    return _CACHE["nc"]


def kernel(**inputs):
    in_maps = host_prep(inputs)
    nc = _get_nc()
    res = run_bass_kernel_spmd(nc, in_maps, core_ids=list(range(M)))
    return combine(res.results)
